# revision 10
# baseline (speedup 1.0000x reference)
"""Trainium2 Bass kernel for nn_Decoder: fused single-step LSTM decoder.

Reference computation (per token t of batch b, state never advances):
    gates = x[b,t] @ W_ih.T + (h0[b] @ W_hh.T + b_ih + b_hh)     # [2048]
    i, f, g, o = sigmoid/sigmoid/tanh/sigmoid of gate quarters
    c = f * c0[b] + i * g
    h = o * tanh(c)
    out[b,t] = h @ fc_w.T + fc_b                                 # [513]

Sharding: data-parallel, batch 64 -> 8 batches per core on 8 NeuronCores.

Per-core design ("fp8 DoubleRow gates, feature-major"):
  - One supertile = one batch = 1024 tokens. 8 supertiles per core.
  - x is transposed to feature-major fp8 e4m3 on the host; the kernel
    does straight DMAs only (no xbar transposes).
  - gates are computed transposed (gatesT[g-chunk, tok]) with fp8
    DoubleRow matmuls: each DR matmul contracts 2 k-tiles of 128
    features at 0.5 PE cycles/output element (4x bf16 throughput).
    Accuracy: W_ih (x64) is split into an fp8 "hi" part over features
    0..511 plus an fp8 "lo" part over the SHIFTED window 1..512 that
    holds the quantization residual for features 1..511 and the full
    column for feature 512 (so no separate K=1 rank-1 matmul; only
    features 0 and 512 are single-fp8, which is negligible).
    Per 128-gate chunk and 512-token half: 4 DR matmuls.
  - The per-batch bias (h0 @ W_hh.T + b_ih + b_hh, fp32 on host) and
    the 1/64 descale ride the ScalarE activation (func(in*scale+bias));
    activations are merged over the whole supertile: 16 ops of
    [128,1024] + 4 tanh(c). ScalarE is the roofline of this kernel.
  - LSTM cell feature-major: GPSIMD does t1 = i*g (bf16); DVE does the
    fused c = f*c0 + t1 (scalar_tensor_tensor) and h = o*tanh(c) in
    bf16; ScalarE does tanh(c).
  - fc in bf16 (h bf16 is the DR-free lhsT): 4 K-chunk matmuls per
    (128-token, 258-col half); no on-device bias - fc_b is added in
    fp32 on the host after the gather (exact and frees PE cycles).
    DVE copies PSUM->SBUF; DMA out per 128-token subtile.
  - fc for supertile s is emitted interleaved into supertile s+1's
    gate chunk loop so ScalarE never starves during the fc phase.
"""

from contextlib import ExitStack

import ml_dtypes
import numpy as np

import concourse.bass as bass
import concourse.tile as tile
from concourse import bacc, mybir
from concourse.bass_utils import run_bass_kernel_spmd

FP32 = mybir.dt.float32
BF16 = mybir.dt.bfloat16
FP8 = mybir.dt.float8e4
AFT = mybir.ActivationFunctionType
ALU = mybir.AluOpType
DRSW = mybir.MatmulPerfMode.DoubleRowSwInterleave

N_CORES = 8
B, T, D = 64, 1024, 513
H = 512
B_LOC = B // N_CORES  # 8 batches per core
TOK = B_LOC * T  # 8192 tokens per core
ST = T  # supertile = one batch = 1024 tokens
NST = TOK // ST  # 8 supertiles
DPAD = 516  # fc output padded width
NHALF = 258  # fc N split halves (each fits one PSUM bank)

SW = 64.0  # W_ih fp8 scale (PSUM = 64*gates)
E4 = ml_dtypes.float8_e4m3
BFD = ml_dtypes.bfloat16


def build_nc(reps=1, mode="full"):
    nc = bacc.Bacc("TRN2", target_bir_lowering=False, debug=False, num_devices=N_CORES)
    xt = nc.dram_tensor("xt", [513, TOK], FP8, kind="ExternalInput").ap()
    whi = nc.dram_tensor("whi", [128, 2, 2048, 2], FP8, kind="ExternalInput").ap()
    wlo = nc.dram_tensor("wlo", [128, 2, 2048, 2], FP8, kind="ExternalInput").ap()
    fcw = nc.dram_tensor("fcw", [128, 4, DPAD], BF16, kind="ExternalInput").ap()
    bct = nc.dram_tensor("bconst", [128, 16 * B_LOC], FP32, kind="ExternalInput").ap()
    c0t = nc.dram_tensor("c0t", [128, 4 * B_LOC], FP32, kind="ExternalInput").ap()
    out = nc.dram_tensor("out", [TOK, D], FP32, kind="ExternalOutput").ap()

    with tile.TileContext(nc) as tc, ExitStack() as ctx:
        const = ctx.enter_context(tc.tile_pool(name="const", bufs=1))
        xp = ctx.enter_context(tc.tile_pool(name="xp", bufs=12))
        sigp = ctx.enter_context(tc.tile_pool(name="sig", bufs=8))
        tmpp = ctx.enter_context(tc.tile_pool(name="tmp", bufs=3))
        hp = ctx.enter_context(tc.tile_pool(name="h", bufs=12))
        outp = ctx.enter_context(tc.tile_pool(name="osb", bufs=4))
        gpp = ctx.enter_context(tc.tile_pool(name="gp", bufs=2, space="PSUM"))
        fmp = ctx.enter_context(tc.tile_pool(name="fm", bufs=2, space="PSUM"))

        def emit_x(st):
            """DMA supertile st's x: hi pairs (feats 0..511) and lo pairs
            (shifted window, feats 1..512), fp8 DR-pair layout."""
            ts = st * ST
            tiles = []
            for off, nm in ((0, "xh"), (1, "xl")):
                for p in range(2):
                    t = xp.tile([128, 2, ST], FP8, tag="xpair", name=f"{nm}{p}")
                    for i in range(2):
                        r = off + p * 256 + i * 128
                        nc.sync.dma_start(t[:, i, :], xt[r : r + 128, ts : ts + ST])
                    tiles.append(t)
            return tiles

        # ---- startup: first supertiles' x, then weights ----
        x_queue = [emit_x(0)]

        whi_sb = const.tile([128, 2, 2048, 2], FP8, tag="whi")
        wlo_sb = const.tile([128, 2, 2048, 2], FP8, tag="wlo")
        nc.sync.dma_start(whi_sb[:], whi)
        nc.sync.dma_start(wlo_sb[:], wlo)
        bct_sb = const.tile([128, 16 * B_LOC], FP32, tag="bct")
        c0_sb = const.tile([128, 4 * B_LOC], FP32, tag="c0")
        nc.sync.dma_start(bct_sb[:], bct)
        nc.sync.dma_start(c0_sb[:], c0t)
        fcw_sb = const.tile([128, 4, DPAD], BF16, tag="fcw")
        nc.sync.dma_start(fcw_sb[:], fcw)

        x_queue.append(emit_x(1))
        x_queue.append(emit_x(2 % NST))

        gate_funcs = [AFT.Sigmoid, AFT.Sigmoid, AFT.Tanh, AFT.Sigmoid]

        def emit_k_group(k, b, xtiles):
            """Gates + cell for h-chunk k of batch b. Returns h tile."""
            xh0, xh1, xl0, xl1 = xtiles
            gs = []
            for gi in range(4):
                c = gi * 4 + k
                cs = slice(c * 128, (c + 1) * 128)
                gp = gpp.tile([128, 1024], FP32, tag="gp")
                # weight-major order: both token halves of a weight slice are
                # adjacent so the PE can reuse/overlap the weight load
                mmspec = [
                    (whi_sb[:, 0, cs, :], xh0),
                    (whi_sb[:, 1, cs, :], xh1),
                    (wlo_sb[:, 0, cs, :], xl0),
                    (wlo_sb[:, 1, cs, :], xl1),
                ]
                for wi, (w, xt) in enumerate(mmspec):
                    for hf in range(2):
                        hs = slice(hf * 512, (hf + 1) * 512)
                        nc.tensor.matmul(gp[:, hs], w, xt[:, :, hs],
                                         start=(wi == 0), stop=(wi == 3),
                                         perf_mode=DRSW)
                if mode == "mmonly":
                    continue
                s = sigp.tile([128, 1024], BF16, tag="sig")
                nc.scalar.activation(
                    s[:], gp[:], gate_funcs[gi],
                    bias=bct_sb[:, c * B_LOC + b : c * B_LOC + b + 1],
                    scale=1.0 / SW,
                )
                gs.append(s)
            if mode in ("mmonly", "noact"):
                return None
            i_s, f_s, g_s, o_s = gs
            t1 = tmpp.tile([128, 1024], BF16, tag="t1")
            nc.gpsimd.tensor_mul(t1[:], i_s[:], g_s[:])
            cc = tmpp.tile([128, 1024], FP32, tag="cc")
            nc.vector.scalar_tensor_tensor(
                cc[:], in0=f_s[:],
                scalar=c0_sb[:, k * B_LOC + b : k * B_LOC + b + 1],
                in1=t1[:], op0=ALU.mult, op1=ALU.add,
            )
            th = tmpp.tile([128, 1024], BF16, tag="th")
            nc.scalar.activation(th[:], cc[:], AFT.Tanh)
            h = hp.tile([128, 1024], BF16, tag="h")
            nc.vector.tensor_mul(h[:], o_s[:], th[:])
            return h

        def emit_fc(st, hn, m):
            """fc for 128-token subtile m of supertile st (bf16, no bias)."""
            if mode != "full":
                return
            ts = st * ST
            msl = slice(m * 128, (m + 1) * 128)
            pf = fmp.tile([128, 1024], FP32, tag="fm")
            for kd in range(4):
                for nh in range(2):
                    po = pf[:, nh * 512 : nh * 512 + NHALF]
                    nsl = slice(nh * NHALF, (nh + 1) * NHALF)
                    nc.tensor.matmul(po, hn[kd][:, msl], fcw_sb[:, kd, nsl],
                                     start=(kd == 0), stop=(kd == 3))
            osb = outp.tile([128, 2, NHALF], FP32, tag="osb")
            nc.vector.tensor_copy(
                osb[:],
                pf[:].rearrange("p (a z) -> p a z", a=2)[:, :, 0:NHALF],
            )
            nc.sync.dma_start(
                out[ts + m * 128 : ts + (m + 1) * 128, :],
                osb[:].rearrange("p a z -> p (a z)")[:, 0:D],
            )

        # ---- prologue: gates+cell for supertile 0 (no fc yet) ----
        xtiles = x_queue.pop(0)
        h_prev = [emit_k_group(k, 0, xtiles) for k in range(4)]

        # ---- main loop: fc(st) interleaved with gates+cell(st+1) ----
        rep_ctx = tc.For_i(0, reps, 1) if reps > 1 else None
        if rep_ctx is not None:
            rep_ctx.__enter__()
        for st in range(NST):
            s_next = (st + 1) % NST
            do_gates = (reps > 1) or (st < NST - 1)
            if do_gates:
                xtiles = x_queue.pop(0)
                if reps > 1:
                    x_queue.append(emit_x((s_next + 2) % NST))
                elif s_next + 2 < NST:
                    x_queue.append(emit_x(s_next + 2))
            h_new = []
            for k in range(4):
                if do_gates:
                    h_new.append(emit_k_group(k, s_next, xtiles))
                emit_fc(st, h_prev, 2 * k)
                emit_fc(st, h_prev, 2 * k + 1)
            if do_gates:
                h_prev = h_new
        if rep_ctx is not None:
            rep_ctx.__exit__(None, None, None)

    nc.compile()
    return nc


_NC_CACHE = []


def get_nc():
    if not _NC_CACHE:
        _NC_CACHE.append(build_nc())
    return _NC_CACHE[0]


def make_in_maps(decoder_inputs, h0, c0, W_ih, W_hh, b_ih, b_hh, fc_w, fc_b):
    di = np.asarray(decoder_inputs, dtype=np.float32)
    h0 = np.asarray(h0, dtype=np.float32)[0]  # [64, 512]
    c0 = np.asarray(c0, dtype=np.float32)[0]
    W_ih = np.asarray(W_ih, dtype=np.float32)
    W_hh = np.asarray(W_hh, dtype=np.float32)
    b_ih = np.asarray(b_ih, dtype=np.float32)
    b_hh = np.asarray(b_hh, dtype=np.float32)
    fc_w = np.asarray(fc_w, dtype=np.float32)

    bc = h0 @ W_hh.T + b_ih + b_hh  # [64, 2048]

    # W_ih x64 fp8: hi = features 0..511; lo = shifted window 1..512
    # (residual for 1..511, full fp8 column for feature 512).
    W_s = SW * W_ih  # [2048, 513]
    W_hi8 = W_s[:, 0:512].astype(E4)
    lo_slots = np.concatenate(
        [W_s[:, 1:512] - W_hi8[:, 1:512].astype(np.float32), W_s[:, 512:513]],
        axis=1,
    )
    W_lo8 = lo_slots.astype(E4)  # [2048, 512]

    def wpack(w8):
        # SwInterleave layout: arr[k, pair, c*128+j, i] = w8[c*128+(127-j),
        # pair*256 + i*128 + k]  -> [128, 2(pair), 2048(g), 2(ktile)]
        tmp = w8.reshape(16, 128, 2, 2, 128)  # (c, m, pair, i, k)
        return np.ascontiguousarray(
            np.flip(tmp, axis=1).transpose(4, 2, 0, 1, 3).reshape(128, 2, 2048, 2))

    whi_a = wpack(W_hi8)
    wlo_a = wpack(W_lo8)

    # fc weights bf16, [128, 4(k-chunk), 516]
    fc_pad = np.zeros((512, DPAD), dtype=BFD)
    fc_pad[:, 0:D] = fc_w.T.astype(BFD)
    fcw_a = np.ascontiguousarray(
        fc_pad.reshape(4, 128, DPAD).transpose(1, 0, 2))

    in_maps = []
    for core in range(N_CORES):
        bs = core * B_LOC
        xc = di[bs : bs + B_LOC].reshape(TOK, D)
        xt_a = np.ascontiguousarray(xc.T.astype(E4))  # [513, TOK]
        bct = np.ascontiguousarray(
            bc[bs : bs + B_LOC]
            .reshape(B_LOC, 16, 128)
            .transpose(2, 1, 0)
            .reshape(128, -1)
        )
        c0c = np.ascontiguousarray(
            c0[bs : bs + B_LOC]
            .reshape(B_LOC, 4, 128)
            .transpose(2, 1, 0)
            .reshape(128, -1)
        )
        in_maps.append(
            {
                "xt": xt_a,
                "whi": whi_a,
                "wlo": wlo_a,
                "fcw": fcw_a,
                "bconst": bct,
                "c0t": c0c,
            }
        )
    return in_maps


def kernel(**inputs):
    in_maps = make_in_maps(**inputs)
    nc = get_nc()
    res = run_bass_kernel_spmd(nc, in_maps, core_ids=list(range(N_CORES)))
    out = np.concatenate([res.results[c]["out"] for c in range(N_CORES)], axis=0)
    out = out.reshape(B, T, D)
    out += np.asarray(inputs["fc_b"], dtype=np.float32)  # exact fp32 bias
    return out


# revision 12
# speedup vs baseline: 1.0075x; 1.0075x over previous
"""Trainium2 Bass kernel for nn_Decoder: fused single-step LSTM decoder.

Reference computation (per token t of batch b, state never advances):
    gates = x[b,t] @ W_ih.T + (h0[b] @ W_hh.T + b_ih + b_hh)     # [2048]
    i, f, g, o = sigmoid/sigmoid/tanh/sigmoid of gate quarters
    c = f * c0[b] + i * g
    h = o * tanh(c)
    out[b,t] = h @ fc_w.T + fc_b                                 # [513]

Sharding: data-parallel, batch 64 -> 8 batches per core on 8 NeuronCores.

Per-core design ("fp8 DoubleRowSwInterleave gates, feature-major"):
  - One supertile = one batch = 1024 tokens. 8 supertiles per core.
  - x is transposed to feature-major fp8 e4m3 on the host; the kernel
    does straight DMAs only (no xbar transposes).
  - gates are computed transposed (gatesT[g-chunk, tok]) with fp8
    DoubleRowSwInterleave matmuls: each contracts 2 k-tiles of 128
    features per pass (measured on HW: ~268-312ns vs ~262ns for a
    bf16 K=128 matmul, i.e. ~1.8x contraction throughput; plain
    DoubleRow is slower, ~390ns, and bf16 K=1 matmuls cost ~465ns).
    Accuracy: W_ih (x64) is split into an fp8 "hi" part over features
    0..511 plus an fp8 "lo" part over the SHIFTED window 1..512 that
    holds the quantization residual for features 1..511 and the full
    column for feature 512 - so no K=1 rank-1 matmul is needed; only
    features 0 and 512 are single-fp8 (negligible). Per 128-gate
    chunk and 512-token half: 4 DRSW matmuls. 128 matmuls/supertile.
  - The per-batch bias (h0 @ W_hh.T + b_ih + b_hh, fp32 on host) and
    the 1/64 descale ride the ScalarE activation (func(in*scale+bias));
    activations are merged over the whole supertile: 16 ops of
    [128,1024] + 4 tanh(c), fully hidden under the PE.
  - LSTM cell feature-major: GPSIMD does t1 = i*g (bf16); DVE does the
    fused c = f*c0 + t1 (scalar_tensor_tensor) and h = o*tanh(c) in
    bf16; ScalarE does tanh(c). All hidden under the PE.
  - fc in bf16 (h bf16 is the lhsT): 4 K-chunk matmuls per (128-token,
    258-col half); no on-device bias - fc_b is added in fp32 on the
    host after the gather. DVE copies PSUM->SBUF; DMA out per subtile.
  - fc for supertile s is emitted interleaved into supertile s+1's
    gate chunk loop; matmuls are weight-major so both token halves of
    a weight slice are adjacent.
  - Measured: PE matmul issue (stream + serial LDWEIGHTS + ~30ns fix
    per matmul) is the critical path; everything else overlaps.
"""

from contextlib import ExitStack

import ml_dtypes
import numpy as np

import concourse.bass as bass
import concourse.tile as tile
from concourse import bacc, mybir
from concourse.bass_utils import run_bass_kernel_spmd

FP32 = mybir.dt.float32
BF16 = mybir.dt.bfloat16
FP8 = mybir.dt.float8e4
AFT = mybir.ActivationFunctionType
ALU = mybir.AluOpType
DRSW = mybir.MatmulPerfMode.DoubleRowSwInterleave

N_CORES = 8
B, T, D = 64, 1024, 513
H = 512
B_LOC = B // N_CORES  # 8 batches per core
TOK = B_LOC * T  # 8192 tokens per core
ST = T  # supertile = one batch = 1024 tokens
NST = TOK // ST  # 8 supertiles
DPAD = 516  # fc output padded width
NHALF = 258  # fc N split halves (each fits one PSUM bank)

SW = 64.0  # W_ih fp8 scale (PSUM = 64*gates)
E4 = ml_dtypes.float8_e4m3
BFD = ml_dtypes.bfloat16


def build_nc(reps=1, mode="full"):
    nc = bacc.Bacc("TRN2", target_bir_lowering=False, debug=False, num_devices=N_CORES)
    xt = nc.dram_tensor("xt", [513, TOK], FP8, kind="ExternalInput").ap()
    whi = nc.dram_tensor("whi", [128, 2, 2048, 2], FP8, kind="ExternalInput").ap()
    wlo = nc.dram_tensor("wlo", [128, 2, 2048, 2], FP8, kind="ExternalInput").ap()
    fcw = nc.dram_tensor("fcw", [128, 4, DPAD], BF16, kind="ExternalInput").ap()
    bct = nc.dram_tensor("bconst", [128, 16 * B_LOC], FP32, kind="ExternalInput").ap()
    c0t = nc.dram_tensor("c0t", [128, 4 * B_LOC], FP32, kind="ExternalInput").ap()
    out = nc.dram_tensor("out", [TOK, D], FP32, kind="ExternalOutput").ap()

    with tile.TileContext(nc) as tc, ExitStack() as ctx:
        const = ctx.enter_context(tc.tile_pool(name="const", bufs=1))
        xp = ctx.enter_context(tc.tile_pool(name="xp", bufs=12))
        sigp = ctx.enter_context(tc.tile_pool(name="sig", bufs=8))
        tmpp = ctx.enter_context(tc.tile_pool(name="tmp", bufs=3))
        hp = ctx.enter_context(tc.tile_pool(name="h", bufs=12))
        outp = ctx.enter_context(tc.tile_pool(name="osb", bufs=4))
        gpp = ctx.enter_context(tc.tile_pool(name="gp", bufs=2, space="PSUM"))
        fmp = ctx.enter_context(tc.tile_pool(name="fm", bufs=2, space="PSUM"))

        def emit_x(st):
            """DMA supertile st's x: hi pairs (feats 0..511) and lo pairs
            (shifted window, feats 1..512), fp8 DR-pair layout."""
            ts = st * ST
            tiles = []
            for off, nm in ((0, "xh"), (1, "xl")):
                for p in range(2):
                    t = xp.tile([128, 2, ST], FP8, tag="xpair", name=f"{nm}{p}")
                    for i in range(2):
                        r = off + p * 256 + i * 128
                        nc.sync.dma_start(t[:, i, :], xt[r : r + 128, ts : ts + ST])
                    tiles.append(t)
            return tiles

        # ---- startup: first supertiles' x, then weights ----
        x_queue = [emit_x(0)]

        whi_sb = const.tile([128, 2, 2048, 2], FP8, tag="whi")
        wlo_sb = const.tile([128, 2, 2048, 2], FP8, tag="wlo")
        nc.sync.dma_start(whi_sb[:], whi)
        nc.sync.dma_start(wlo_sb[:], wlo)
        bct_sb = const.tile([128, 16 * B_LOC], FP32, tag="bct")
        c0_sb = const.tile([128, 4 * B_LOC], FP32, tag="c0")
        nc.sync.dma_start(bct_sb[:], bct)
        nc.sync.dma_start(c0_sb[:], c0t)
        fcw_sb = const.tile([128, 4, DPAD], BF16, tag="fcw")
        nc.sync.dma_start(fcw_sb[:], fcw)

        x_queue.append(emit_x(1))
        x_queue.append(emit_x(2 % NST))

        gate_funcs = [AFT.Sigmoid, AFT.Sigmoid, AFT.Tanh, AFT.Sigmoid]

        def emit_k_group(k, b, xtiles):
            """Gates + cell for h-chunk k of batch b. Returns h tile."""
            xh0, xh1, xl0, xl1 = xtiles
            gs = []
            for gi in range(4):
                c = gi * 4 + k
                cs = slice(c * 128, (c + 1) * 128)
                gp = gpp.tile([128, 1024], FP32, tag="gp")
                # weight-major order: both token halves of a weight slice are
                # adjacent so the PE can reuse/overlap the weight load
                mmspec = [
                    (whi_sb[:, 0, cs, :], xh0),
                    (whi_sb[:, 1, cs, :], xh1),
                    (wlo_sb[:, 0, cs, :], xl0),
                    (wlo_sb[:, 1, cs, :], xl1),
                ]
                for wi, (w, xt) in enumerate(mmspec):
                    for hf in range(2):
                        hs = slice(hf * 512, (hf + 1) * 512)
                        nc.tensor.matmul(gp[:, hs], w, xt[:, :, hs],
                                         start=(wi == 0), stop=(wi == 3),
                                         perf_mode=DRSW)
                if mode == "mmonly":
                    continue
                s = sigp.tile([128, 1024], BF16, tag="sig")
                nc.scalar.activation(
                    s[:], gp[:], gate_funcs[gi],
                    bias=bct_sb[:, c * B_LOC + b : c * B_LOC + b + 1],
                    scale=1.0 / SW,
                )
                gs.append(s)
            if mode in ("mmonly", "noact"):
                return None
            i_s, f_s, g_s, o_s = gs
            t1 = tmpp.tile([128, 1024], BF16, tag="t1")
            nc.gpsimd.tensor_mul(t1[:], i_s[:], g_s[:])
            cc = tmpp.tile([128, 1024], FP32, tag="cc")
            nc.vector.scalar_tensor_tensor(
                cc[:], in0=f_s[:],
                scalar=c0_sb[:, k * B_LOC + b : k * B_LOC + b + 1],
                in1=t1[:], op0=ALU.mult, op1=ALU.add,
            )
            th = tmpp.tile([128, 1024], BF16, tag="th")
            nc.scalar.activation(th[:], cc[:], AFT.Tanh)
            h = hp.tile([128, 1024], BF16, tag="h")
            nc.vector.tensor_mul(h[:], o_s[:], th[:])
            return h

        def emit_fc(st, hn, m):
            """fc for 128-token subtile m of supertile st (bf16, no bias)."""
            if mode != "full":
                return
            ts = st * ST
            msl = slice(m * 128, (m + 1) * 128)
            pf = fmp.tile([128, 1024], FP32, tag="fm")
            for kd in range(4):
                for nh in range(2):
                    po = pf[:, nh * 512 : nh * 512 + NHALF]
                    nsl = slice(nh * NHALF, (nh + 1) * NHALF)
                    nc.tensor.matmul(po, hn[kd][:, msl], fcw_sb[:, kd, nsl],
                                     start=(kd == 0), stop=(kd == 3))
            osb = outp.tile([128, 2, NHALF], FP32, tag="osb")
            nc.vector.tensor_copy(
                osb[:],
                pf[:].rearrange("p (a z) -> p a z", a=2)[:, :, 0:NHALF],
            )
            nc.sync.dma_start(
                out[ts + m * 128 : ts + (m + 1) * 128, :],
                osb[:].rearrange("p a z -> p (a z)")[:, 0:D],
            )

        # ---- prologue: gates+cell for supertile 0 (no fc yet) ----
        xtiles = x_queue.pop(0)
        h_prev = [emit_k_group(k, 0, xtiles) for k in range(4)]

        # ---- main loop: fc(st) interleaved with gates+cell(st+1) ----
        rep_ctx = (
            tc.For_i(0, reps, 1, staggered_reset=True) if reps > 1 else None
        )
        if rep_ctx is not None:
            rep_ctx.__enter__()
        for st in range(NST):
            s_next = (st + 1) % NST
            do_gates = (reps > 1) or (st < NST - 1)
            if do_gates:
                xtiles = x_queue.pop(0)
                if reps > 1:
                    x_queue.append(emit_x((s_next + 2) % NST))
                elif s_next + 2 < NST:
                    x_queue.append(emit_x(s_next + 2))
            h_new = []
            for k in range(4):
                if do_gates:
                    h_new.append(emit_k_group(k, s_next, xtiles))
                emit_fc(st, h_prev, 2 * k)
                emit_fc(st, h_prev, 2 * k + 1)
            if do_gates:
                h_prev = h_new
        if rep_ctx is not None:
            rep_ctx.__exit__(None, None, None)

    nc.compile()
    return nc


_NC_CACHE = []


def get_nc():
    if not _NC_CACHE:
        _NC_CACHE.append(build_nc())
    return _NC_CACHE[0]


def make_in_maps(decoder_inputs, h0, c0, W_ih, W_hh, b_ih, b_hh, fc_w, fc_b):
    di = np.asarray(decoder_inputs, dtype=np.float32)
    h0 = np.asarray(h0, dtype=np.float32)[0]  # [64, 512]
    c0 = np.asarray(c0, dtype=np.float32)[0]
    W_ih = np.asarray(W_ih, dtype=np.float32)
    W_hh = np.asarray(W_hh, dtype=np.float32)
    b_ih = np.asarray(b_ih, dtype=np.float32)
    b_hh = np.asarray(b_hh, dtype=np.float32)
    fc_w = np.asarray(fc_w, dtype=np.float32)

    bc = h0 @ W_hh.T + b_ih + b_hh  # [64, 2048]

    # W_ih x64 fp8: hi = features 0..511; lo = shifted window 1..512
    # (residual for 1..511, full fp8 column for feature 512).
    W_s = SW * W_ih  # [2048, 513]
    W_hi8 = W_s[:, 0:512].astype(E4)
    lo_slots = np.concatenate(
        [W_s[:, 1:512] - W_hi8[:, 1:512].astype(np.float32), W_s[:, 512:513]],
        axis=1,
    )
    W_lo8 = lo_slots.astype(E4)  # [2048, 512]

    def wpack(w8):
        # SwInterleave layout: arr[k, pair, c*128+j, i] = w8[c*128+(127-j),
        # pair*256 + i*128 + k]  -> [128, 2(pair), 2048(g), 2(ktile)]
        tmp = w8.reshape(16, 128, 2, 2, 128)  # (c, m, pair, i, k)
        return np.ascontiguousarray(
            np.flip(tmp, axis=1).transpose(4, 2, 0, 1, 3).reshape(128, 2, 2048, 2))

    whi_a = wpack(W_hi8)
    wlo_a = wpack(W_lo8)

    # fc weights bf16, [128, 4(k-chunk), 516]
    fc_pad = np.zeros((512, DPAD), dtype=BFD)
    fc_pad[:, 0:D] = fc_w.T.astype(BFD)
    fcw_a = np.ascontiguousarray(
        fc_pad.reshape(4, 128, DPAD).transpose(1, 0, 2))

    in_maps = []
    for core in range(N_CORES):
        bs = core * B_LOC
        xc = di[bs : bs + B_LOC].reshape(TOK, D)
        xt_a = np.ascontiguousarray(xc.T.astype(E4))  # [513, TOK]
        bct = np.ascontiguousarray(
            bc[bs : bs + B_LOC]
            .reshape(B_LOC, 16, 128)
            .transpose(2, 1, 0)
            .reshape(128, -1)
        )
        c0c = np.ascontiguousarray(
            c0[bs : bs + B_LOC]
            .reshape(B_LOC, 4, 128)
            .transpose(2, 1, 0)
            .reshape(128, -1)
        )
        in_maps.append(
            {
                "xt": xt_a,
                "whi": whi_a,
                "wlo": wlo_a,
                "fcw": fcw_a,
                "bconst": bct,
                "c0t": c0c,
            }
        )
    return in_maps


def kernel(**inputs):
    in_maps = make_in_maps(**inputs)
    nc = get_nc()
    res = run_bass_kernel_spmd(nc, in_maps, core_ids=list(range(N_CORES)))
    out = np.concatenate([res.results[c]["out"] for c in range(N_CORES)], axis=0)
    out = out.reshape(B, T, D)
    out += np.asarray(inputs["fc_b"], dtype=np.float32)  # exact fp32 bias
    return out
